# revision 31
# baseline (speedup 1.0000x reference)
"""Trainium2 Bass kernel for a pre-LN transformer block (B=4, T=2048, C=512, H=8).

Sharding: 8 cores, 2 per batch element. Core group g takes q-chunks {2i+g}
(256 tokens each), causal k-extents padded to the uniform schedule
{512, 1024, 1536, 2048}; padding + the causal diagonal are neutralized by
multiplicative {0,1} masks over the last 4 k-tiles of each slot (per-core
data, slot-invariant).

All matmul operands are bf16 (fp32 PSUM accumulate): halves DMA + SBUF and
enables FWL fast weight loads. x (xkT, all T tokens) and every weight stay
resident in SBUF — nothing is re-streamed from HBM. LN means are folded
into QKV projections as K=1 rank-1 corrections; per-token rstd is fused
into PSUM-evacuation multiplies; the FFN mean-subtraction is fused into the
xnewT -> bf16 cast and rstd2 commutes past the ReLU to the FF2 evac.
Attention: S^T = K_h^T q with 2 heads per 128-row pass (row-groups in
separate PSUM banks), exp on ACT, AV via token-major V augmented with a
ones column so the softmax denominator falls out of the same matmul; both
head accumulators share one PSUM bank as a single accumulation group.
Per-slot tail (Wo + residual + LN2 stats + FFN) is emitted right after each
slot's attention so PE-bound FFN work overlaps ACT-bound softmax of the
next slot.
"""

import os
import sys

sys.path.insert(0, "/opt/trn_rl_repo")

import contextlib

import numpy as np
import ml_dtypes

import functools

import concourse.bass as bass
import concourse.tile as tile
from concourse import bacc, mybir
from concourse.bass_utils import run_bass_kernel_spmd

# Prefer the activation-table set that holds BOTH ln and exp so the per-slot
# LN2 rstd chain (Ln/Exp) and the attention softmax (Exp) resolve to one set
# — otherwise the table-load pass alternates exp_and_others / natural_log and
# pays ~2.7us per switch, 18 times.
_orig_get_act_tables = bacc.get_activation_tables


@functools.cache
def _act_tables_ln_exp_first(arch):
    # Keep the canonical set order (act_func_set_id is an index into it) but
    # hide exp/ln from the single-function sets, so the load-insertion pass
    # can only satisfy them via the combined set.
    tabs = _orig_get_act_tables(arch)
    combined = "natural_log_exp_and_others"
    if combined not in tabs:
        return tabs
    out = {}
    for k, v in tabs.items():
        if k != combined:
            v = v - {mybir.ActivationFunctionType.Exp,
                     mybir.ActivationFunctionType.Ln}
        out[k] = v
    return out


bacc.get_activation_tables = _act_tables_ln_exp_first

P = 128
C = 512
T = 2048
TQ = 1024
H = 8
HS = 64
F = 2048
NS = 4            # c-subtiles of C
NSLOT = 4         # q-chunks (slots) per core, 256 tokens each
QC = 256          # q-chunk width
EXTS = [512, 1024, 1536, 2048]   # scheduled k-extent per slot
EPS = 1e-5

f32 = mybir.dt.float32
bf16 = mybir.dt.bfloat16
AF = mybir.ActivationFunctionType
ALU = mybir.AluOpType

_last_exec_time_ns = None
_last_results = None


def _build_program(limit="full"):
    nc = bacc.Bacc(name="block")

    def inp(name, shape, dt=bf16):
        return nc.declare_dram_parameter(name, list(shape), dt, isOutput=False)

    xkT = inp("xkT", (C, T))          # x[b].T, bf16
    xqT = inp("xqT", (C, TQ))         # q-chunk columns of x[b].T, slot order
    wqT = inp("wqT", (C, C))          # (Wq*g1).T * C^-0.5
    wkT = inp("wkT", (C, C))
    wvT = inp("wvT", (C, C))
    woT = inp("woT", (C, C))
    w1T = inp("w1T", (C, F))          # (W_ff1*g2).T
    w2T = inp("w2T", (F, C))
    nwqcs = inp("nwqcs", (1, C))      # -colsum(wqT)
    nwkcs = inp("nwkcs", (1, C))
    nwvcs = inp("nwvcs", (1, C))
    wocs = inp("wocs", (P, NS))       # colsum_j Wo[j, c'] as column tiles
    masks = inp("masks", (P, 4, QC))  # last-4 kt masks (slot-invariant)
    yT = nc.declare_dram_parameter("yT", [C, TQ], f32, isOutput=True)
    scr = nc.dram_tensor("scratch_rk", [1, T], f32)

    NKT = [e // P for e in EXTS]

    def _body(tc, top):
        # ---------- persistent constants ----------
        pc = top.enter_context(tc.tile_pool(name="const", bufs=1))
        ones_r = pc.tile([P, 1], bf16, tag="ones_r")
        nc.vector.memset(ones_r, 1.0)
        eps_sb = pc.tile([1, 1], f32, tag="eps")
        nc.vector.memset(eps_sb, EPS)

        # ---------- persistent data (whole kernel) ----------
        px = top.enter_context(tc.tile_pool(name="pX", bufs=1))
        xk_sb = px.tile([P, NS, T], bf16, tag="xkT")        # 16KB
        for s in range(NS):
            for th in range(2):
                nc.sync.dma_start(
                    out=xk_sb[:, s, th * 1024:(th + 1) * 1024],
                    in_=xkT.ap()[s * P:(s + 1) * P, th * 1024:(th + 1) * 1024])
        xq_sb = px.tile([P, NS, TQ], bf16, tag="xqT")       # 8KB
        for s in range(NS):
            nc.sync.dma_start(out=xq_sb[:, s], in_=xqT.ap()[s * P:(s + 1) * P, :])
        mask_sb = px.tile([P, 4, QC], bf16, tag="masks")    # 2KB
        nc.sync.dma_start(out=mask_sb, in_=masks.ap())

        pw = top.enter_context(tc.tile_pool(name="pW", bufs=1))
        wk_sb = pw.tile([P, NS, C], bf16, tag="wk")
        wv_sb = pw.tile([P, NS, C], bf16, tag="wv")
        wq_sb = pw.tile([P, NS, C], bf16, tag="wq")
        wo_sb = pw.tile([P, NS, C], bf16, tag="wo")
        for t_sb, src in ((wk_sb, wkT), (wv_sb, wvT), (wq_sb, wqT), (wo_sb, woT)):
            nc.sync.dma_start(out=t_sb, in_=src.ap().rearrange("(s p) c -> p s c", p=P))
        w1_sb = pw.tile([P, NS, F], bf16, tag="w1")
        nc.sync.dma_start(out=w1_sb, in_=w1T.ap().rearrange("(s p) c -> p s c", p=P))
        w2_sb = pw.tile([P, F // P, C], bf16, tag="w2")
        nc.sync.dma_start(out=w2_sb, in_=w2T.ap().rearrange("(s p) c -> p s c", p=P))
        nwqcs_sb = pw.tile([1, C], bf16, tag="nwqcs")
        nc.sync.dma_start(out=nwqcs_sb, in_=nwqcs.ap())
        nwkcs_sb = pw.tile([1, C], bf16, tag="nwkcs")
        nc.sync.dma_start(out=nwkcs_sb, in_=nwkcs.ap())
        nwvcs_sb = pw.tile([1, C], bf16, tag="nwvcs")
        nc.sync.dma_start(out=nwvcs_sb, in_=nwvcs.ap())
        wocs_sb = pw.tile([P, NS], bf16, tag="wocs")
        nc.sync.dma_start(out=wocs_sb, in_=wocs.ap())

        # persistent small rows
        pr = top.enter_context(tc.tile_pool(name="pRows", bufs=1))
        muq_bf = pr.tile([1, TQ], bf16, tag="muq")           # 2KB

        # stats rows live only through the projections
        es_rows = contextlib.ExitStack()
        prs = es_rows.enter_context(tc.tile_pool(name="pRowsTmp", bufs=1, side="right"))
        muk_bf = prs.tile([1, T], bf16, tag="muk")           # 4KB
        rkb_sb = prs.tile([P, T], f32, tag="rkb")            # 8KB
        rqb_sb = prs.tile([P, TQ], f32, tag="rqb")           # 4KB
        rk_col = prs.tile([P, T // P], f32, tag="rkcol")     # 64B

        # K/Q/V outputs + attention output (persistent)
        pk = top.enter_context(tc.tile_pool(name="pKQV", bufs=1))
        kT_sb = pk.tile([P, NS, T], bf16, tag="kT")          # 16KB
        vaug_sb = pk.tile([P, T // P, H * 65], bf16, tag="vaug")  # 16.25KB
        nc.vector.memset(
            vaug_sb.rearrange("p t (h x) -> p t h x", x=65)[:, :, :, 64:65], 1.0)
        qT_sb = pk.tile([P, NS, TQ], bf16, tag="qT")         # 8KB
        attnT_sb = pk.tile([P, NS, TQ], bf16, tag="attnT")   # 8KB

        # tail persistents
        pt_ = top.enter_context(tc.tile_pool(name="pTailP", bufs=1))
        xnewT_sb = pt_.tile([P, NS, TQ], f32, tag="xnewT")   # 16KB
        xnewTr_sb = pt_.tile([P, NS, TQ], bf16, tag="xnewTr")  # 8KB

        # small rotating pools
        prow = top.enter_context(tc.tile_pool(name="pRowPool", bufs=2))
        px2 = top.enter_context(tc.tile_pool(name="pX2", bufs=2))
        ppt = top.enter_context(tc.tile_pool(name="pPt", bufs=3))
        prre = top.enter_context(tc.tile_pool(name="pRre", bufs=2))

        # PSUM pools: pstat(2) + pap(2) + sp(4) = 8 banks; pstat closes ->
        # po(2) opens; pap closes -> tail(2) opens.
        es_ap = contextlib.ExitStack()
        pap = es_ap.enter_context(
            tc.tile_pool(name="pApPs", bufs=2, space="PSUM", side="left"))
        es_stat = contextlib.ExitStack()
        pstat = es_stat.enter_context(
            tc.tile_pool(name="pStPs", bufs=2, space="PSUM", side="left"))

        # ---------- LN1 stats over all T (k side) + q side ----------
        # NOTE: the Tile scheduler does NOT preserve per-engine program order,
        # so two accumulation groups must never share a PSUM bank — a late
        # start=True would clear the sibling group's has_written bits.
        def stats_chunks(x_sb, n_tok, mu_row, rb_sb, key, scr_off):
            for u in range(n_tok // C):
                sl = slice(u * C, (u + 1) * C)
                stx = pstat.tile([1, C], f32, tag="stx", name=f"stx{key}_{u}")
                st2 = pstat.tile([1, C], f32, tag="st2", name=f"st2{key}_{u}")
                for s in range(NS):
                    nc.tensor.matmul(stx, ones_r, x_sb[:, s, sl],
                                     start=(s == 0), stop=(s == NS - 1))
                for s in range(NS):
                    x2 = px2.tile([P, C], bf16, tag="x2", name=f"x2{key}_{u}_{s}")
                    nc.vector.tensor_tensor(out=x2, in0=x_sb[:, s, sl],
                                            in1=x_sb[:, s, sl], op=ALU.mult)
                    nc.tensor.matmul(st2, ones_r, x2,
                                     start=(s == 0), stop=(s == NS - 1))
                nc.vector.tensor_scalar_mul(out=mu_row[:, sl], in0=stx,
                                            scalar1=1.0 / C)
                e2 = prow.tile([1, C], f32, tag="e2", name=f"e2{key}_{u}")
                nc.vector.tensor_scalar_mul(out=e2, in0=st2, scalar1=1.0 / C)
                m2 = prow.tile([1, C], f32, tag="m2", name=f"m2{key}_{u}")
                nc.vector.tensor_tensor(out=m2, in0=mu_row[:, sl],
                                        in1=mu_row[:, sl], op=ALU.mult)
                nc.vector.tensor_tensor(out=e2, in0=e2, in1=m2, op=ALU.subtract)
                nc.scalar.activation(out=e2, in_=e2, func=AF.Ln, bias=eps_sb)
                rs = prow.tile([1, C], f32, tag="rs", name=f"rs{key}_{u}")
                nc.scalar.activation(out=rs, in_=e2, func=AF.Exp, scale=-0.5)
                nc.gpsimd.partition_broadcast(rb_sb[:, sl], rs)
                if scr_off is not None:
                    nc.sync.dma_start(out=scr.ap()[:, scr_off + u * C:
                                                   scr_off + (u + 1) * C], in_=rs)

        stats_chunks(xk_sb, T, muk_bf, rkb_sb, "k", 0)
        stats_chunks(xq_sb, TQ, muq_bf, rqb_sb, "q", None)
        # token-major rstd column for the V evac (DRAM round trip: partition-
        # scatter sbuf->sbuf DMA corrupts data on HW)
        nc.sync.dma_start(out=rk_col, in_=scr.ap().rearrange("a (o p) -> (a p) o", p=P))

        es_stat.close()   # frees 4 banks -> psp
        psp = top.enter_context(
            tc.tile_pool(name="pSpPs", bufs=2, space="PSUM", side="right"))

        if limit == "stats":
            nc.sync.dma_start(out=yT.ap()[0:P, 0:T // P], in_=rk_col)
            es_ap.close()
            es_rows.close()
            return

        # ---------- projections: K (all T), V (all T), Q (q chunks) ----------
        for j in range(NS):
            for tch in range(T // C):
                sl = slice(tch * C, (tch + 1) * C)
                ps = pap.tile([P, C], f32, tag="proj", name=f"k{j}_{tch}")
                for s in range(NS):
                    nc.tensor.matmul(ps, wk_sb[:, s, j * P:(j + 1) * P],
                                     xk_sb[:, s, sl], start=(s == 0), stop=False)
                nc.tensor.matmul(ps, nwkcs_sb[:, j * P:(j + 1) * P], muk_bf[:, sl],
                                 start=False, stop=True)
                nc.vector.tensor_tensor(out=kT_sb[:, j, sl], in0=ps,
                                        in1=rkb_sb[:, sl], op=ALU.mult)
        for tt in range(T // P):
            tsl = slice(tt * P, (tt + 1) * P)
            ps = pap.tile([P, C], f32, tag="proj", name=f"v{tt}")
            for s in range(NS):
                nc.tensor.matmul(ps, xk_sb[:, s, tsl], wv_sb[:, s, :],
                                 start=(s == 0), stop=False)
            nc.tensor.matmul(ps, muk_bf[:, tsl], nwvcs_sb,
                             start=False, stop=True)
            nc.vector.tensor_scalar_mul(
                out=vaug_sb[:, tt].rearrange("p (h x) -> p h x", x=65)[:, :, 0:HS],
                in0=ps.rearrange("p (h d) -> p h d", d=HS),
                scalar1=rk_col[:, tt:tt + 1])
        for j in range(NS):
            for tch in range(TQ // C):
                sl = slice(tch * C, (tch + 1) * C)
                ps = pap.tile([P, C], f32, tag="proj", name=f"q{j}_{tch}")
                for s in range(NS):
                    nc.tensor.matmul(ps, wq_sb[:, s, j * P:(j + 1) * P],
                                     xq_sb[:, s, sl], start=(s == 0), stop=False)
                nc.tensor.matmul(ps, nwqcs_sb[:, j * P:(j + 1) * P],
                                 muq_bf[:, sl], start=False, stop=True)
                nc.vector.tensor_tensor(out=qT_sb[:, j, sl], in0=ps,
                                        in1=rqb_sb[:, sl], op=ALU.mult)

        ppo = top.enter_context(
            tc.tile_pool(name="pPoPs", bufs=1, space="PSUM", side="right"))

        if limit == "proj":
            for s in range(NS):
                nc.sync.dma_start(out=yT.ap()[s * P:(s + 1) * P, 0:512],
                                  in_=qT_sb[:, s].bitcast(f32))
            dbg = pk.tile([P, 16, 8], bf16, tag="dbgones")
            nc.vector.tensor_copy(
                out=dbg,
                in_=vaug_sb.rearrange("p t (h x) -> p t h x", x=65)[:, :, :, 64])
            nc.sync.dma_start(out=yT.ap()[0:P, 512:576], in_=dbg.bitcast(f32))
            nc.sync.dma_start(out=yT.ap()[P:2 * P, 512:772],
                              in_=vaug_sb[:, 0].bitcast(f32))
            nc.sync.dma_start(out=yT.ap()[3 * P:4 * P, 512:1024],
                              in_=kT_sb[:, 0, 0:1024].bitcast(f32))
            es_ap.close()
            es_rows.close()
            return

        # ---------- attention + per-slot tail ----------
        ptl = pnrm = pat = pyt = None

        def tail(slot):
            qsl = slice(slot * QC, (slot + 1) * QC)
            # Wo + residual
            for j in range(NS):
                ps = ptl.tile([P, C], f32, tag="tl", name=f"wo{slot}_{j}")
                for s in range(NS):
                    nc.tensor.matmul(ps[:, 0:QC], wo_sb[:, s, j * P:(j + 1) * P],
                                     attnT_sb[:, s, qsl], start=(s == 0),
                                     stop=(s == NS - 1), skip_group_check=True)
                nc.vector.tensor_tensor(out=xnewT_sb[:, j, qsl], in0=ps[:, 0:QC],
                                        in1=xq_sb[:, j, qsl], op=ALU.add)
            # LN2 stats: mean via muq + wocs@attnT; var via ones@xnew^2
            # (separate banks: interleaved accum groups must not share one)
            stm = ptl.tile([P, C], f32, tag="tl", name=f"st2m_{slot}")
            stv = ptl.tile([P, C], f32, tag="tl", name=f"st2v_{slot}")
            for s in range(NS):
                nc.tensor.matmul(stm[0:1, 0:QC], wocs_sb[:, s:s + 1],
                                 attnT_sb[:, s, qsl], start=(s == 0),
                                 stop=(s == NS - 1), skip_group_check=True)
            for s in range(NS):
                x2 = px2.tile([P, QC], bf16, tag="x2n", name=f"x2n{slot}_{s}")
                nc.vector.tensor_tensor(out=x2, in0=xnewT_sb[:, s, qsl],
                                        in1=xnewT_sb[:, s, qsl], op=ALU.mult)
                nc.tensor.matmul(stv[0:1, 0:QC], ones_r, x2,
                                 start=(s == 0), stop=(s == NS - 1),
                                 skip_group_check=True)
            mu2 = prow.tile([1, QC], f32, tag="mu2", name=f"mu2_{slot}")
            tmp = prow.tile([1, QC], f32, tag="t2a", name=f"t2a_{slot}")
            nc.vector.tensor_scalar_mul(out=tmp, in0=stm[0:1, 0:QC], scalar1=1.0 / C)
            nc.vector.tensor_tensor(out=mu2, in0=tmp, in1=muq_bf[:, qsl], op=ALU.add)
            e2 = prow.tile([1, QC], f32, tag="e2t", name=f"e2t_{slot}")
            nc.vector.tensor_scalar_mul(out=e2, in0=stv[0:1, 0:QC], scalar1=1.0 / C)
            nc.vector.tensor_tensor(out=tmp, in0=mu2, in1=mu2, op=ALU.mult)
            nc.vector.tensor_tensor(out=e2, in0=e2, in1=tmp, op=ALU.subtract)
            nc.scalar.activation(out=e2, in_=e2, func=AF.Ln, bias=eps_sb)
            rs = prow.tile([1, QC], f32, tag="rst", name=f"rs2_{slot}")
            nc.scalar.activation(out=rs, in_=e2, func=AF.Exp, scale=-0.5)
            r2b = pnrm.tile([P, QC], f32, tag="r2b", name=f"r2b_{slot}")
            nc.gpsimd.partition_broadcast(r2b, rs)
            mu2b = pnrm.tile([P, QC], f32, tag="mu2b", name=f"mu2b_{slot}")
            nc.gpsimd.partition_broadcast(mu2b, mu2)
            for j in range(NS):
                nc.vector.tensor_tensor(out=xnewTr_sb[:, j, qsl],
                                        in0=xnewT_sb[:, j, qsl], in1=mu2b,
                                        op=ALU.subtract)
            # FFN
            aT = pat.tile([P, F // P, QC], bf16, tag="aT", name=f"aT{slot}")
            for fj in range(F // P):
                ps = ptl.tile([P, C], f32, tag="tl", name=f"ff1_{slot}_{fj}")
                for s in range(NS):
                    nc.tensor.matmul(ps[:, 0:QC], w1_sb[:, s, fj * P:(fj + 1) * P],
                                     xnewTr_sb[:, s, qsl], start=(s == 0),
                                     stop=(s == NS - 1), skip_group_check=True)
                if fj % 2 == 0:
                    nc.scalar.activation(out=aT[:, fj], in_=ps[:, 0:QC],
                                         func=AF.Relu)
                else:
                    nc.vector.tensor_scalar_max(out=aT[:, fj], in0=ps[:, 0:QC],
                                                scalar1=0.0)
            for j in range(NS):
                ps = ptl.tile([P, C], f32, tag="tl", name=f"ff2_{slot}_{j}")
                for fj in range(F // P):
                    nc.tensor.matmul(ps[:, 0:QC], w2_sb[:, fj, j * P:(j + 1) * P],
                                     aT[:, fj], start=(fj == 0),
                                     stop=(fj == F // P - 1), skip_group_check=True)
                yt = pyt.tile([P, QC], f32, tag="yt", name=f"y{slot}_{j}")
                nc.vector.tensor_tensor(out=yt, in0=ps[:, 0:QC], in1=r2b, op=ALU.mult)
                nc.vector.tensor_tensor(out=yt, in0=yt, in1=xnewT_sb[:, j, qsl],
                                        op=ALU.add)
                nc.sync.dma_start(out=yT.ap()[j * P:(j + 1) * P, qsl], in_=yt)

        for slot in range(NSLOT):
            nkt = NKT[slot]
            qsl = slice(slot * QC, (slot + 1) * QC)
            for jj in range(NS):
                # one accumulator bank per head: the Tile scheduler reorders
                # tensor-engine instructions, so interleaved accumulation
                # groups must not share a bank
                po = [ppo.tile([65, QC], f32, tag=f"av{hi}",
                               name=f"av{slot}_{jj}_{hi}")
                      for hi in range(2)]
                pending = None

                def emit_av(ktp, p_tile, po=po, nkt=nkt, jj=jj):
                    for i in range(2):
                        kt = 2 * ktp + i
                        for hi in range(2):
                            nc.tensor.matmul(
                                po[hi],
                                vaug_sb[:, kt, (2 * jj + hi) * 65:(2 * jj + hi + 1) * 65],
                                p_tile[:, hi, i, :],
                                start=(kt == 0),
                                stop=(kt == nkt - 1),
                            )

                for ktp in range(nkt // 2):
                    # [P, hi, kt-parity, QC]: each bank hosts a single PE
                    # row-group (base-0 / base-64 matmuls must not share)
                    sp = psp.tile([P, 2, 2, QC], f32, tag="sp",
                                  name=f"s{slot}_{jj}_{ktp}")
                    for i in range(2):
                        kt = 2 * ktp + i
                        ksl = slice(kt * P, (kt + 1) * P)
                        nc.tensor.matmul(sp[:, 0, i, :], kT_sb[0:HS, jj, ksl],
                                         qT_sb[0:HS, jj, qsl], start=True, stop=True)
                        nc.tensor.matmul(sp[:, 1, i, :], kT_sb[HS:P, jj, ksl],
                                         qT_sb[HS:P, jj, qsl], start=True, stop=True)
                    pt = ppt.tile([P, 2, 2, QC], bf16, tag="p",
                                  name=f"p{slot}_{jj}_{ktp}")
                    nc.scalar.activation(out=pt, in_=sp, func=AF.Exp)
                    for i in range(2):
                        kt = 2 * ktp + i
                        if kt >= nkt - 4:
                            m = mask_sb[:, kt - (nkt - 4)]
                            for hi in range(2):
                                eng = nc.vector if hi == 0 else nc.gpsimd
                                eng.tensor_tensor(out=pt[:, hi, i, :],
                                                  in0=pt[:, hi, i, :],
                                                  in1=m, op=ALU.mult)
                    if pending is not None:
                        emit_av(*pending)
                    pending = (ktp, pt)
                emit_av(*pending)

                for hi in range(2):
                    # reciprocal_approx_* misreads PSUM operands — stage the
                    # denominator row through SBUF first (in-place recip)
                    r_row = prow.tile([1, QC], f32, tag="rr", name=f"r{slot}_{jj}_{hi}")
                    nc.vector.tensor_copy(out=r_row, in_=po[hi][64:65, :])
                    nc.vector.reciprocal_approx_fast(out=r_row, in_=r_row)
                    rrep = prre.tile([HS, QC], f32, tag="rrep",
                                     name=f"rrb{slot}_{jj}_{hi}")
                    nc.gpsimd.partition_broadcast(rrep, r_row)
                    nc.vector.tensor_tensor(
                        out=attnT_sb[hi * HS:(hi + 1) * HS, jj, qsl],
                        in0=po[hi][0:HS, :], in1=rrep, op=ALU.mult)

            if slot == 0:
                es_ap.close()   # frees 2 banks -> tail pool
                es_rows.close()  # frees stats rows -> tail SBUF pools
                ptl = top.enter_context(
                    tc.tile_pool(name="pTlPs", bufs=2, space="PSUM", side="right"))
                pnrm = top.enter_context(tc.tile_pool(name="pNrm", bufs=2, side="right"))
                pat = top.enter_context(tc.tile_pool(name="pAT", bufs=1, side="right"))
                pyt = top.enter_context(tc.tile_pool(name="pYt", bufs=2, side="right"))
            if limit == "attn":
                continue
            tail(slot)

        if limit == "attn":
            for s in range(NS):
                nc.sync.dma_start(out=yT.ap()[s * P:(s + 1) * P, 0:512],
                                  in_=attnT_sb[:, s].bitcast(f32))
            return

    with tile.TileContext(nc) as tc, contextlib.ExitStack() as top:
        _body(tc, top)
    nc.finalize()
    return nc


_prog = None


def _get_program():
    global _prog
    if _prog is None:
        _prog = _build_program(os.environ.get("KPH", "full"))
    return _prog


def _bf(a):
    return np.ascontiguousarray(np.asarray(a, np.float32).astype(ml_dtypes.bfloat16))


def _host_prep(x, Wq, Wk, Wv, Wo, bo, g1, b1, g2, b2, W_ff1, b_ff1, W_ff2, b_ff2):
    x = np.asarray(x, np.float32)
    for nm, v in (("bo", bo), ("b1", b1), ("b2", b2), ("b_ff1", b_ff1), ("b_ff2", b_ff2)):
        if not np.allclose(np.asarray(v), 0.0):
            raise NotImplementedError(f"nonzero bias {nm} not supported")
    g1 = np.asarray(g1, np.float32)
    g2 = np.asarray(g2, np.float32)
    scale = np.float32(np.float64(C) ** -0.5)
    wqT = (np.asarray(Wq) * (g1 * scale)[None, :]).T.astype(np.float32)
    wkT = (np.asarray(Wk) * g1[None, :]).T.astype(np.float32)
    wvT = (np.asarray(Wv) * g1[None, :]).T.astype(np.float32)
    woT = np.asarray(Wo).T.astype(np.float32)
    w1T = (np.asarray(W_ff1) * g2[None, :]).T.astype(np.float32)
    w2T = np.asarray(W_ff2).T.astype(np.float32)
    shared = dict(
        wqT=_bf(wqT), wkT=_bf(wkT), wvT=_bf(wvT), woT=_bf(woT),
        w1T=_bf(w1T), w2T=_bf(w2T),
        nwqcs=_bf(-wqT.sum(0)[None, :]),
        nwkcs=_bf(-wkT.sum(0)[None, :]),
        nwvcs=_bf(-wvT.sum(0)[None, :]),
        wocs=_bf(np.asarray(Wo).sum(0).astype(np.float32).reshape(NS, P).T),
    )
    # masks: slot-invariant. g=1 (ext == E): kr0,1 ones; kr2,3 diag.
    # g=0 (ext == E-256): kr0,1 diag; kr2,3 zero.
    p_i = np.arange(P)[:, None]
    q_i = np.arange(QC)[None, :]
    mg = []
    for g in range(2):
        m = np.zeros((P, 4, QC), np.float32)
        for kr in range(4):
            if g == 0:
                if kr < 2:
                    m[:, kr, :] = (p_i + 128 * kr <= q_i).astype(np.float32)
            else:
                if kr < 2:
                    m[:, kr, :] = 1.0
                else:
                    m[:, kr, :] = (p_i + 128 * (kr - 2) <= q_i).astype(np.float32)
        mg.append(_bf(m))
    in_maps = []
    for core in range(8):
        b, g = core // 2, core % 2
        chunks = [2 * i + g for i in range(NSLOT)]
        qrows = np.concatenate([np.arange(QC * ch, QC * (ch + 1)) for ch in chunks])
        in_maps.append(dict(
            shared,
            xkT=_bf(x[b].T),
            xqT=_bf(x[b][qrows].T),
            masks=mg[g],
        ))
    return in_maps


def kernel(**inputs):
    global _last_exec_time_ns, _last_results
    inputs = {k: np.asarray(v) for k, v in inputs.items()}
    in_maps = _host_prep(**inputs)
    nc = _get_program()
    trace = os.environ.get("KERNEL_TRACE", "0") == "1"
    res = run_bass_kernel_spmd(nc, in_maps, list(range(8)), trace=trace)
    _last_exec_time_ns = res.exec_time_ns
    _last_results = res
    out = np.empty((4, T, C), np.float32)
    for core in range(8):
        b, g = core // 2, core % 2
        yt = res.results[core]["yT"]
        for i in range(NSLOT):
            ch = 2 * i + g
            out[b, QC * ch:QC * (ch + 1), :] = yt[:, QC * i:QC * (i + 1)].T
    return out


# revision 32
# speedup vs baseline: 1.8132x; 1.8132x over previous
"""Trainium2 Bass kernel for a pre-LN transformer block (B=4, T=2048, C=512, H=8).

Sharding: 8 cores, 2 per batch element. Core group g takes q-chunks {2i+g}
(256 tokens each), causal k-extents padded to the uniform schedule
{512, 1024, 1536, 2048}; padding + the causal diagonal are neutralized by
multiplicative {0,1} masks over the last 4 k-tiles of each slot (per-core
data, slot-invariant).

All matmul operands are bf16 (fp32 PSUM accumulate): halves DMA + SBUF and
enables FWL fast weight loads. x (xkT, all T tokens) and every weight stay
resident in SBUF — nothing is re-streamed from HBM. LN means are folded
into QKV projections as K=1 rank-1 corrections; per-token rstd is fused
into PSUM-evacuation multiplies; the FFN mean-subtraction is fused into the
xnewT -> bf16 cast and rstd2 commutes past the ReLU to the FF2 evac.
Attention: S^T = K_h^T q with 2 heads per 128-row pass (row-groups in
separate PSUM banks), exp on ACT, AV via token-major V augmented with a
ones column so the softmax denominator falls out of the same matmul; both
head accumulators share one PSUM bank as a single accumulation group.
Per-slot tail (Wo + residual + LN2 stats + FFN) is emitted right after each
slot's attention so PE-bound FFN work overlaps ACT-bound softmax of the
next slot.
"""

import os
import sys

sys.path.insert(0, "/opt/trn_rl_repo")

import contextlib

import numpy as np
import ml_dtypes

import functools

import concourse.bass as bass
import concourse.tile as tile
from concourse import bacc, mybir
from concourse.bass_utils import run_bass_kernel_spmd

# Prefer the activation-table set that holds BOTH ln and exp so the per-slot
# LN2 rstd chain (Ln/Exp) and the attention softmax (Exp) resolve to one set
# — otherwise the table-load pass alternates exp_and_others / natural_log and
# pays ~2.7us per switch, 18 times.
_orig_get_act_tables = bacc.get_activation_tables


@functools.cache
def _act_tables_ln_exp_first(arch):
    # Keep the canonical set order (act_func_set_id is an index into it) but
    # hide exp/ln from the single-function sets, so the load-insertion pass
    # can only satisfy them via the combined set.
    tabs = _orig_get_act_tables(arch)
    combined = "natural_log_exp_and_others"
    if combined not in tabs:
        return tabs
    out = {}
    for k, v in tabs.items():
        if k != combined:
            v = v - {mybir.ActivationFunctionType.Exp,
                     mybir.ActivationFunctionType.Ln}
        out[k] = v
    return out


bacc.get_activation_tables = _act_tables_ln_exp_first

P = 128
C = 512
T = 2048
TQ = 1024
H = 8
HS = 64
F = 2048
NS = 4            # c-subtiles of C
NSLOT = 4         # q-chunks (slots) per core, 256 tokens each
QC = 256          # q-chunk width
EXTS = [512, 1024, 1536, 2048]   # scheduled k-extent per slot
EPS = 1e-5

f32 = mybir.dt.float32
bf16 = mybir.dt.bfloat16
AF = mybir.ActivationFunctionType
ALU = mybir.AluOpType

_last_exec_time_ns = None
_last_results = None


def _build_program(limit="full"):
    nc = bacc.Bacc(name="block")

    def inp(name, shape, dt=bf16):
        return nc.declare_dram_parameter(name, list(shape), dt, isOutput=False)

    xkT = inp("xkT", (C, T))          # x[b].T, bf16
    xqT = inp("xqT", (C, TQ))         # q-chunk columns of x[b].T, slot order
    wqT = inp("wqT", (C, C))          # (Wq*g1).T * C^-0.5
    wkT = inp("wkT", (C, C))
    wvT = inp("wvT", (C, C))
    woT = inp("woT", (C, C))
    w1T = inp("w1T", (C, F))          # (W_ff1*g2).T
    w2T = inp("w2T", (F, C))
    nwqcs = inp("nwqcs", (1, C))      # -colsum(wqT)
    nwkcs = inp("nwkcs", (1, C))
    nwvcs = inp("nwvcs", (1, C))
    wocs = inp("wocs", (P, NS))       # colsum_j Wo[j, c'] as column tiles
    masks = inp("masks", (P, 4, QC))  # last-4 kt masks (slot-invariant)
    yT = nc.declare_dram_parameter("yT", [C, TQ], f32, isOutput=True)
    scr = nc.dram_tensor("scratch_rk", [1, T], f32)

    NKT = [e // P for e in EXTS]

    def _body(tc, top):
        # ---------- persistent constants ----------
        pc = top.enter_context(tc.tile_pool(name="const", bufs=1))
        ones_r = pc.tile([P, 1], bf16, tag="ones_r")
        nc.vector.memset(ones_r, 1.0)
        eps_sb = pc.tile([1, 1], f32, tag="eps")
        nc.vector.memset(eps_sb, EPS)

        # ---------- persistent data (whole kernel) ----------
        px = top.enter_context(tc.tile_pool(name="pX", bufs=1))
        xk_sb = px.tile([P, NS, T], bf16, tag="xkT")        # 16KB
        for s in range(NS):
            for th in range(2):
                nc.sync.dma_start(
                    out=xk_sb[:, s, th * 1024:(th + 1) * 1024],
                    in_=xkT.ap()[s * P:(s + 1) * P, th * 1024:(th + 1) * 1024])
        xq_sb = px.tile([P, NS, TQ], bf16, tag="xqT")       # 8KB
        for s in range(NS):
            nc.sync.dma_start(out=xq_sb[:, s], in_=xqT.ap()[s * P:(s + 1) * P, :])
        mask_sb = px.tile([P, 4, QC], bf16, tag="masks")    # 2KB
        nc.sync.dma_start(out=mask_sb, in_=masks.ap())

        pw = top.enter_context(tc.tile_pool(name="pW", bufs=1))
        wk_sb = pw.tile([P, NS, C], bf16, tag="wk")
        wv_sb = pw.tile([P, NS, C], bf16, tag="wv")
        wq_sb = pw.tile([P, NS, C], bf16, tag="wq")
        wo_sb = pw.tile([P, NS, C], bf16, tag="wo")
        for t_sb, src in ((wk_sb, wkT), (wv_sb, wvT), (wq_sb, wqT), (wo_sb, woT)):
            nc.sync.dma_start(out=t_sb, in_=src.ap().rearrange("(s p) c -> p s c", p=P))
        w1_sb = pw.tile([P, NS, F], bf16, tag="w1")
        nc.sync.dma_start(out=w1_sb, in_=w1T.ap().rearrange("(s p) c -> p s c", p=P))
        w2_sb = pw.tile([P, F // P, C], bf16, tag="w2")
        nc.sync.dma_start(out=w2_sb, in_=w2T.ap().rearrange("(s p) c -> p s c", p=P))
        nwqcs_sb = pw.tile([1, C], bf16, tag="nwqcs")
        nc.sync.dma_start(out=nwqcs_sb, in_=nwqcs.ap())
        nwkcs_sb = pw.tile([1, C], bf16, tag="nwkcs")
        nc.sync.dma_start(out=nwkcs_sb, in_=nwkcs.ap())
        nwvcs_sb = pw.tile([1, C], bf16, tag="nwvcs")
        nc.sync.dma_start(out=nwvcs_sb, in_=nwvcs.ap())
        wocs_sb = pw.tile([P, NS], bf16, tag="wocs")
        nc.sync.dma_start(out=wocs_sb, in_=wocs.ap())

        # persistent small rows
        pr = top.enter_context(tc.tile_pool(name="pRows", bufs=1))
        muq_bf = pr.tile([1, TQ], bf16, tag="muq")           # 2KB

        # stats rows live only through the projections
        es_rows = contextlib.ExitStack()
        prs = es_rows.enter_context(tc.tile_pool(name="pRowsTmp", bufs=1, side="right"))
        muk_bf = prs.tile([1, T], bf16, tag="muk")           # 4KB
        rkb_sb = prs.tile([P, T], f32, tag="rkb")            # 8KB
        rqb_sb = prs.tile([P, TQ], f32, tag="rqb")           # 4KB
        rk_col = prs.tile([P, T // P], f32, tag="rkcol")     # 64B

        # K/Q/V outputs + attention output (persistent)
        pk = top.enter_context(tc.tile_pool(name="pKQV", bufs=1))
        kT_sb = pk.tile([P, NS, T], bf16, tag="kT")          # 16KB
        vaug_sb = pk.tile([P, T // P, H * 65], bf16, tag="vaug")  # 16.25KB
        nc.vector.memset(
            vaug_sb.rearrange("p t (h x) -> p t h x", x=65)[:, :, :, 64:65], 1.0)
        qT_sb = pk.tile([P, NS, TQ], bf16, tag="qT")         # 8KB
        attnT_sb = pk.tile([P, NS, TQ], bf16, tag="attnT")   # 8KB

        # tail persistents
        pt_ = top.enter_context(tc.tile_pool(name="pTailP", bufs=1))
        xnewT_sb = pt_.tile([P, NS, TQ], f32, tag="xnewT")   # 16KB
        xnewTr_sb = pt_.tile([P, NS, TQ], bf16, tag="xnewTr")  # 8KB

        # small rotating pools
        prow = top.enter_context(tc.tile_pool(name="pRowPool", bufs=2))
        px2 = top.enter_context(tc.tile_pool(name="pX2", bufs=2))
        ppt = top.enter_context(tc.tile_pool(name="pPt", bufs=3))
        prre = top.enter_context(tc.tile_pool(name="pRre", bufs=2))

        # PSUM pools: pstat(2) + pap(2) + sp(4) = 8 banks; pstat closes ->
        # po(2) opens; pap closes -> tail(2) opens.
        es_ap = contextlib.ExitStack()
        pap = es_ap.enter_context(
            tc.tile_pool(name="pApPs", bufs=2, space="PSUM", side="left"))
        es_stat = contextlib.ExitStack()
        pstat = es_stat.enter_context(
            tc.tile_pool(name="pStPs", bufs=2, space="PSUM", side="left"))

        # ---------- LN1 stats over all T (k side) + q side ----------
        # NOTE: the Tile scheduler does NOT preserve per-engine program order,
        # so two accumulation groups must never share a PSUM bank — a late
        # start=True would clear the sibling group's has_written bits.
        def stats_chunks(x_sb, n_tok, mu_row, rb_sb, key, scr_off):
            for u in range(n_tok // C):
                sl = slice(u * C, (u + 1) * C)
                stx = pstat.tile([1, C], f32, tag="stx", name=f"stx{key}_{u}")
                st2 = pstat.tile([1, C], f32, tag="st2", name=f"st2{key}_{u}")
                for s in range(NS):
                    nc.tensor.matmul(stx, ones_r, x_sb[:, s, sl],
                                     start=(s == 0), stop=(s == NS - 1))
                for s in range(NS):
                    x2 = px2.tile([P, C], bf16, tag="x2", name=f"x2{key}_{u}_{s}")
                    nc.vector.tensor_tensor(out=x2, in0=x_sb[:, s, sl],
                                            in1=x_sb[:, s, sl], op=ALU.mult)
                    nc.tensor.matmul(st2, ones_r, x2,
                                     start=(s == 0), stop=(s == NS - 1))
                nc.vector.tensor_scalar_mul(out=mu_row[:, sl], in0=stx,
                                            scalar1=1.0 / C)
                e2 = prow.tile([1, C], f32, tag="e2", name=f"e2{key}_{u}")
                nc.vector.tensor_scalar_mul(out=e2, in0=st2, scalar1=1.0 / C)
                m2 = prow.tile([1, C], f32, tag="m2", name=f"m2{key}_{u}")
                nc.vector.tensor_tensor(out=m2, in0=mu_row[:, sl],
                                        in1=mu_row[:, sl], op=ALU.mult)
                nc.vector.tensor_tensor(out=e2, in0=e2, in1=m2, op=ALU.subtract)
                nc.scalar.activation(out=e2, in_=e2, func=AF.Ln, bias=eps_sb)
                rs = prow.tile([1, C], f32, tag="rs", name=f"rs{key}_{u}")
                nc.scalar.activation(out=rs, in_=e2, func=AF.Exp, scale=-0.5)
                nc.gpsimd.partition_broadcast(rb_sb[:, sl], rs)
                if scr_off is not None:
                    nc.sync.dma_start(out=scr.ap()[:, scr_off + u * C:
                                                   scr_off + (u + 1) * C], in_=rs)

        stats_chunks(xk_sb, T, muk_bf, rkb_sb, "k", 0)
        stats_chunks(xq_sb, TQ, muq_bf, rqb_sb, "q", None)
        # token-major rstd column for the V evac (DRAM round trip: partition-
        # scatter sbuf->sbuf DMA corrupts data on HW)
        nc.sync.dma_start(out=rk_col, in_=scr.ap().rearrange("a (o p) -> (a p) o", p=P))

        es_stat.close()   # frees 4 banks -> psp
        psp = top.enter_context(
            tc.tile_pool(name="pSpPs", bufs=2, space="PSUM", side="right"))

        if limit == "stats":
            nc.sync.dma_start(out=yT.ap()[0:P, 0:T // P], in_=rk_col)
            es_ap.close()
            es_rows.close()
            return

        # ---------- projections: K (all T), V (all T), Q (q chunks) ----------
        for j in range(NS):
            for tch in range(T // C):
                sl = slice(tch * C, (tch + 1) * C)
                ps = pap.tile([P, C], f32, tag="proj", name=f"k{j}_{tch}")
                for s in range(NS):
                    nc.tensor.matmul(ps, wk_sb[:, s, j * P:(j + 1) * P],
                                     xk_sb[:, s, sl], start=(s == 0), stop=False)
                nc.tensor.matmul(ps, nwkcs_sb[:, j * P:(j + 1) * P], muk_bf[:, sl],
                                 start=False, stop=True)
                nc.vector.tensor_tensor(out=kT_sb[:, j, sl], in0=ps,
                                        in1=rkb_sb[:, sl], op=ALU.mult)
        for tt in range(T // P):
            tsl = slice(tt * P, (tt + 1) * P)
            ps = pap.tile([P, C], f32, tag="proj", name=f"v{tt}")
            for s in range(NS):
                nc.tensor.matmul(ps, xk_sb[:, s, tsl], wv_sb[:, s, :],
                                 start=(s == 0), stop=False)
            nc.tensor.matmul(ps, muk_bf[:, tsl], nwvcs_sb,
                             start=False, stop=True)
            nc.vector.tensor_scalar_mul(
                out=vaug_sb[:, tt].rearrange("p (h x) -> p h x", x=65)[:, :, 0:HS],
                in0=ps.rearrange("p (h d) -> p h d", d=HS),
                scalar1=rk_col[:, tt:tt + 1])
        for j in range(NS):
            for tch in range(TQ // C):
                sl = slice(tch * C, (tch + 1) * C)
                ps = pap.tile([P, C], f32, tag="proj", name=f"q{j}_{tch}")
                for s in range(NS):
                    nc.tensor.matmul(ps, wq_sb[:, s, j * P:(j + 1) * P],
                                     xq_sb[:, s, sl], start=(s == 0), stop=False)
                nc.tensor.matmul(ps, nwqcs_sb[:, j * P:(j + 1) * P],
                                 muq_bf[:, sl], start=False, stop=True)
                nc.vector.tensor_tensor(out=qT_sb[:, j, sl], in0=ps,
                                        in1=rqb_sb[:, sl], op=ALU.mult)

        ppo = top.enter_context(
            tc.tile_pool(name="pPoPs", bufs=1, space="PSUM", side="right"))

        if limit == "proj":
            for s in range(NS):
                nc.sync.dma_start(out=yT.ap()[s * P:(s + 1) * P, 0:512],
                                  in_=qT_sb[:, s].bitcast(f32))
            dbg = pk.tile([P, 16, 8], bf16, tag="dbgones")
            nc.vector.tensor_copy(
                out=dbg,
                in_=vaug_sb.rearrange("p t (h x) -> p t h x", x=65)[:, :, :, 64])
            nc.sync.dma_start(out=yT.ap()[0:P, 512:576], in_=dbg.bitcast(f32))
            nc.sync.dma_start(out=yT.ap()[P:2 * P, 512:772],
                              in_=vaug_sb[:, 0].bitcast(f32))
            nc.sync.dma_start(out=yT.ap()[3 * P:4 * P, 512:1024],
                              in_=kT_sb[:, 0, 0:1024].bitcast(f32))
            es_ap.close()
            es_rows.close()
            return

        # ---------- attention + per-slot tail ----------
        ptl = pnrm = pat = pyt = None

        def tail(slot):
            qsl = slice(slot * QC, (slot + 1) * QC)
            # Wo + residual
            for j in range(NS):
                ps = ptl.tile([P, C], f32, tag="tl", name=f"wo{slot}_{j}")
                for s in range(NS):
                    nc.tensor.matmul(ps[:, 0:QC], wo_sb[:, s, j * P:(j + 1) * P],
                                     attnT_sb[:, s, qsl], start=(s == 0),
                                     stop=(s == NS - 1), skip_group_check=True)
                nc.vector.tensor_tensor(out=xnewT_sb[:, j, qsl], in0=ps[:, 0:QC],
                                        in1=xq_sb[:, j, qsl], op=ALU.add)
            # LN2 stats: mean via muq + wocs@attnT; var via ones@xnew^2
            # (separate banks: interleaved accum groups must not share one)
            stm = ptl.tile([P, C], f32, tag="tl", name=f"st2m_{slot}")
            stv = ptl.tile([P, C], f32, tag="tl", name=f"st2v_{slot}")
            for s in range(NS):
                nc.tensor.matmul(stm[0:1, 0:QC], wocs_sb[:, s:s + 1],
                                 attnT_sb[:, s, qsl], start=(s == 0),
                                 stop=(s == NS - 1), skip_group_check=True)
            for s in range(NS):
                x2 = px2.tile([P, QC], bf16, tag="x2n", name=f"x2n{slot}_{s}")
                nc.vector.tensor_tensor(out=x2, in0=xnewT_sb[:, s, qsl],
                                        in1=xnewT_sb[:, s, qsl], op=ALU.mult)
                nc.tensor.matmul(stv[0:1, 0:QC], ones_r, x2,
                                 start=(s == 0), stop=(s == NS - 1),
                                 skip_group_check=True)
            mu2 = prow.tile([1, QC], f32, tag="mu2", name=f"mu2_{slot}")
            tmp = prow.tile([1, QC], f32, tag="t2a", name=f"t2a_{slot}")
            nc.vector.tensor_scalar_mul(out=tmp, in0=stm[0:1, 0:QC], scalar1=1.0 / C)
            nc.vector.tensor_tensor(out=mu2, in0=tmp, in1=muq_bf[:, qsl], op=ALU.add)
            e2 = prow.tile([1, QC], f32, tag="e2t", name=f"e2t_{slot}")
            nc.vector.tensor_scalar_mul(out=e2, in0=stv[0:1, 0:QC], scalar1=1.0 / C)
            nc.vector.tensor_tensor(out=tmp, in0=mu2, in1=mu2, op=ALU.mult)
            nc.vector.tensor_tensor(out=e2, in0=e2, in1=tmp, op=ALU.subtract)
            nc.scalar.activation(out=e2, in_=e2, func=AF.Ln, bias=eps_sb)
            rs = prow.tile([1, QC], f32, tag="rst", name=f"rs2_{slot}")
            nc.scalar.activation(out=rs, in_=e2, func=AF.Exp, scale=-0.5)
            r2b = pnrm.tile([P, QC], f32, tag="r2b", name=f"r2b_{slot}")
            nc.gpsimd.partition_broadcast(r2b, rs)
            mu2b = pnrm.tile([P, QC], f32, tag="mu2b", name=f"mu2b_{slot}")
            nc.gpsimd.partition_broadcast(mu2b, mu2)
            for j in range(NS):
                nc.vector.tensor_tensor(out=xnewTr_sb[:, j, qsl],
                                        in0=xnewT_sb[:, j, qsl], in1=mu2b,
                                        op=ALU.subtract)
            # FFN
            aT = pat.tile([P, F // P, QC], bf16, tag="aT", name=f"aT{slot}")
            for fj in range(F // P):
                ps = ptl.tile([P, C], f32, tag="tl", name=f"ff1_{slot}_{fj}")
                for s in range(NS):
                    nc.tensor.matmul(ps[:, 0:QC], w1_sb[:, s, fj * P:(fj + 1) * P],
                                     xnewTr_sb[:, s, qsl], start=(s == 0),
                                     stop=(s == NS - 1), skip_group_check=True)
                if fj % 2 == 0:
                    nc.scalar.activation(out=aT[:, fj], in_=ps[:, 0:QC],
                                         func=AF.Relu)
                else:
                    nc.vector.tensor_scalar_max(out=aT[:, fj], in0=ps[:, 0:QC],
                                                scalar1=0.0)
            for j in range(NS):
                ps = ptl.tile([P, C], f32, tag="tl", name=f"ff2_{slot}_{j}")
                for fj in range(F // P):
                    nc.tensor.matmul(ps[:, 0:QC], w2_sb[:, fj, j * P:(j + 1) * P],
                                     aT[:, fj], start=(fj == 0),
                                     stop=(fj == F // P - 1), skip_group_check=True)
                yt = pyt.tile([P, QC], f32, tag="yt", name=f"y{slot}_{j}")
                nc.vector.tensor_tensor(out=yt, in0=ps[:, 0:QC], in1=r2b, op=ALU.mult)
                nc.vector.tensor_tensor(out=yt, in0=yt, in1=xnewT_sb[:, j, qsl],
                                        op=ALU.add)
                nc.sync.dma_start(out=yT.ap()[j * P:(j + 1) * P, qsl], in_=yt)

        for slot in range(NSLOT):
            nkt = NKT[slot]
            qsl = slice(slot * QC, (slot + 1) * QC)
            for jj in range(NS):
                # one accumulator bank per head: the Tile scheduler reorders
                # tensor-engine instructions, so interleaved accumulation
                # groups must not share a bank
                po = [ppo.tile([65, QC], f32, tag=f"av{hi}",
                               name=f"av{slot}_{jj}_{hi}")
                      for hi in range(2)]
                pending = None

                def emit_av(ktp, p_tile, po=po, nkt=nkt, jj=jj):
                    for i in range(2):
                        kt = 2 * ktp + i
                        for hi in range(2):
                            nc.tensor.matmul(
                                po[hi],
                                vaug_sb[:, kt, (2 * jj + hi) * 65:(2 * jj + hi + 1) * 65],
                                p_tile[:, hi, i, :],
                                start=(kt == 0),
                                stop=(kt == nkt - 1),
                            )

                for ktp in range(nkt // 2):
                    # [P, hi, kt-parity, QC]: each bank hosts a single PE
                    # row-group (base-0 / base-64 matmuls must not share)
                    sp = psp.tile([P, 2, 2, QC], f32, tag="sp",
                                  name=f"s{slot}_{jj}_{ktp}")
                    for i in range(2):
                        kt = 2 * ktp + i
                        ksl = slice(kt * P, (kt + 1) * P)
                        nc.tensor.matmul(sp[:, 0, i, :], kT_sb[0:HS, jj, ksl],
                                         qT_sb[0:HS, jj, qsl], start=True, stop=True)
                        nc.tensor.matmul(sp[:, 1, i, :], kT_sb[HS:P, jj, ksl],
                                         qT_sb[HS:P, jj, qsl], start=True, stop=True)
                    pt = ppt.tile([P, 2, 2, QC], bf16, tag="p",
                                  name=f"p{slot}_{jj}_{ktp}")
                    nc.scalar.activation(out=pt, in_=sp, func=AF.Exp)
                    for i in range(2):
                        kt = 2 * ktp + i
                        if kt >= nkt - 4:
                            # all masks on DVE: a gpsimd mask op waiting on
                            # exp head-of-line-blocks the rrep broadcasts
                            m = mask_sb[:, kt - (nkt - 4)]
                            for hi in range(2):
                                nc.vector.tensor_tensor(out=pt[:, hi, i, :],
                                                        in0=pt[:, hi, i, :],
                                                        in1=m, op=ALU.mult)
                    if pending is not None:
                        emit_av(*pending)
                    pending = (ktp, pt)
                emit_av(*pending)

                for hi in range(2):
                    # reciprocal_approx_* misreads PSUM operands — stage the
                    # denominator row through SBUF first (in-place recip)
                    r_row = prow.tile([1, QC], f32, tag="rr", name=f"r{slot}_{jj}_{hi}")
                    nc.vector.tensor_copy(out=r_row, in_=po[hi][64:65, :])
                    nc.vector.reciprocal_approx_fast(out=r_row, in_=r_row)
                    rrep = prre.tile([HS, QC], f32, tag="rrep",
                                     name=f"rrb{slot}_{jj}_{hi}")
                    nc.gpsimd.partition_broadcast(rrep, r_row)
                    nc.vector.tensor_tensor(
                        out=attnT_sb[hi * HS:(hi + 1) * HS, jj, qsl],
                        in0=po[hi][0:HS, :], in1=rrep, op=ALU.mult)

            if slot == 0:
                es_ap.close()   # frees 2 banks -> tail pool
                es_rows.close()  # frees stats rows -> tail SBUF pools
                ptl = top.enter_context(
                    tc.tile_pool(name="pTlPs", bufs=2, space="PSUM", side="right"))
                pnrm = top.enter_context(tc.tile_pool(name="pNrm", bufs=2, side="right"))
                pat = top.enter_context(tc.tile_pool(name="pAT", bufs=1, side="right"))
                pyt = top.enter_context(tc.tile_pool(name="pYt", bufs=2, side="right"))
            if limit == "attn":
                continue
            tail(slot)

        if limit == "attn":
            for s in range(NS):
                nc.sync.dma_start(out=yT.ap()[s * P:(s + 1) * P, 0:512],
                                  in_=attnT_sb[:, s].bitcast(f32))
            return

    with tile.TileContext(nc) as tc, contextlib.ExitStack() as top:
        _body(tc, top)
    nc.finalize()
    return nc


_prog = None


def _get_program():
    global _prog
    if _prog is None:
        _prog = _build_program(os.environ.get("KPH", "full"))
    return _prog


def _bf(a):
    return np.ascontiguousarray(np.asarray(a, np.float32).astype(ml_dtypes.bfloat16))


def _host_prep(x, Wq, Wk, Wv, Wo, bo, g1, b1, g2, b2, W_ff1, b_ff1, W_ff2, b_ff2):
    x = np.asarray(x, np.float32)
    for nm, v in (("bo", bo), ("b1", b1), ("b2", b2), ("b_ff1", b_ff1), ("b_ff2", b_ff2)):
        if not np.allclose(np.asarray(v), 0.0):
            raise NotImplementedError(f"nonzero bias {nm} not supported")
    g1 = np.asarray(g1, np.float32)
    g2 = np.asarray(g2, np.float32)
    scale = np.float32(np.float64(C) ** -0.5)
    wqT = (np.asarray(Wq) * (g1 * scale)[None, :]).T.astype(np.float32)
    wkT = (np.asarray(Wk) * g1[None, :]).T.astype(np.float32)
    wvT = (np.asarray(Wv) * g1[None, :]).T.astype(np.float32)
    woT = np.asarray(Wo).T.astype(np.float32)
    w1T = (np.asarray(W_ff1) * g2[None, :]).T.astype(np.float32)
    w2T = np.asarray(W_ff2).T.astype(np.float32)
    shared = dict(
        wqT=_bf(wqT), wkT=_bf(wkT), wvT=_bf(wvT), woT=_bf(woT),
        w1T=_bf(w1T), w2T=_bf(w2T),
        nwqcs=_bf(-wqT.sum(0)[None, :]),
        nwkcs=_bf(-wkT.sum(0)[None, :]),
        nwvcs=_bf(-wvT.sum(0)[None, :]),
        wocs=_bf(np.asarray(Wo).sum(0).astype(np.float32).reshape(NS, P).T),
    )
    # masks: slot-invariant. g=1 (ext == E): kr0,1 ones; kr2,3 diag.
    # g=0 (ext == E-256): kr0,1 diag; kr2,3 zero.
    p_i = np.arange(P)[:, None]
    q_i = np.arange(QC)[None, :]
    mg = []
    for g in range(2):
        m = np.zeros((P, 4, QC), np.float32)
        for kr in range(4):
            if g == 0:
                if kr < 2:
                    m[:, kr, :] = (p_i + 128 * kr <= q_i).astype(np.float32)
            else:
                if kr < 2:
                    m[:, kr, :] = 1.0
                else:
                    m[:, kr, :] = (p_i + 128 * (kr - 2) <= q_i).astype(np.float32)
        mg.append(_bf(m))
    in_maps = []
    for core in range(8):
        b, g = core // 2, core % 2
        chunks = [2 * i + g for i in range(NSLOT)]
        qrows = np.concatenate([np.arange(QC * ch, QC * (ch + 1)) for ch in chunks])
        in_maps.append(dict(
            shared,
            xkT=_bf(x[b].T),
            xqT=_bf(x[b][qrows].T),
            masks=mg[g],
        ))
    return in_maps


def kernel(**inputs):
    global _last_exec_time_ns, _last_results
    inputs = {k: np.asarray(v) for k, v in inputs.items()}
    in_maps = _host_prep(**inputs)
    nc = _get_program()
    trace = os.environ.get("KERNEL_TRACE", "0") == "1"
    res = run_bass_kernel_spmd(nc, in_maps, list(range(8)), trace=trace)
    _last_exec_time_ns = res.exec_time_ns
    _last_results = res
    out = np.empty((4, T, C), np.float32)
    for core in range(8):
        b, g = core // 2, core % 2
        yt = res.results[core]["yT"]
        for i in range(NSLOT):
            ch = 2 * i + g
            out[b, QC * ch:QC * (ch + 1), :] = yt[:, QC * i:QC * (i + 1)].T
    return out
